# revision 1
# baseline (speedup 1.0000x reference)
"""Trainium2 Bass kernel for IntersectionalVolumeRatio.

out[m,n] = exp(sum_d log(softplus(min(Zm,Ze) - max(zm,ze))) - log_men_vol[m])

Math used on device (exp commutes with min/max):
  u      = e^diff = min(e^Zm, e^Ze) * min(e^-zm, e^-ze)
  sp     = ln(1 + u)               = softplus(diff)
  lspq   = ln(sp * (1/softplus(w)))  with w = Zm - zm   (per-mention-dim scale)
  out    = exp(sum_d lspq)         (mention-volume normalization folded in)

Layout per core: partitions = 128 = [d(64) of mention 2j | d(64) of mention 2j+1],
free axis = candidate shard (2500). Reduction over d via PE matmul with a
sliding 0/1 weight window; 64 mention-pairs accumulate into one PSUM tile so
each [128, 500] PSUM tile holds 128 distinct mention rows.
"""

import numpy as np

M = 256
D = 64
N = 20000
NCORES = 8
NS = N // NCORES          # 2500 candidates per core
CH = 500                  # free-dim chunk for PSUM/matmul
NCH = NS // CH

_cache = {}


def _build():
    from concourse import bacc, mybir
    from concourse.tile import TileContext

    F32 = mybir.dt.float32
    F16 = mybir.dt.float16
    AF = mybir.ActivationFunctionType
    OP = mybir.AluOpType

    nc = bacc.Bacc("TRN2", target_bir_lowering=False, debug=False,
                   num_devices=NCORES)
    zedup = nc.dram_tensor("zedup", [128, NS], F32, kind="ExternalInput").ap()
    nzedup = nc.dram_tensor("nzedup", [128, NS], F32, kind="ExternalInput").ap()
    zmc = nc.dram_tensor("zmc", [128, 128], F32, kind="ExternalInput").ap()
    nzmc = nc.dram_tensor("nzmc", [128, 128], F32, kind="ExternalInput").ap()
    out = nc.dram_tensor("out", [M, NS], F32, kind="ExternalOutput").ap()

    with TileContext(nc) as tc:
        with tc.tile_pool(name="persist", bufs=1) as pp, \
             tc.tile_pool(name="work", bufs=3) as wp, \
             tc.tile_pool(name="act", bufs=3) as ap_, \
             tc.tile_pool(name="psum", bufs=1, space="PSUM") as qp:

            # ---- stage inputs ----
            ze_sb = pp.tile([128, NS], F32, tag="ze")
            nze_sb = pp.tile([128, NS], F32, tag="nze")
            zmc_sb = pp.tile([128, 128], F32, tag="zmc")
            nzmc_sb = pp.tile([128, 128], F32, tag="nzmc")
            nc.sync.dma_start(out=ze_sb[:], in_=zedup[:])
            nc.sync.dma_start(out=nze_sb[:], in_=nzedup[:])
            nc.sync.dma_start(out=zmc_sb[:], in_=zmc[:])
            nc.sync.dma_start(out=nzmc_sb[:], in_=nzmc[:])

            # ---- precompute exponentials (device-side) ----
            EZe = pp.tile([128, NS], F32, tag="EZe")
            Enze = pp.tile([128, NS], F32, tag="Enze")
            nc.scalar.activation(EZe[:], ze_sb[:], AF.Exp)
            nc.scalar.activation(Enze[:], nze_sb[:], AF.Exp)
            EZm = pp.tile([128, 128], F32, tag="EZm")
            Enzm = pp.tile([128, 128], F32, tag="Enzm")
            nc.scalar.activation(EZm[:], zmc_sb[:], AF.Exp)
            nc.scalar.activation(Enzm[:], nzmc_sb[:], AF.Exp)

            # ---- per-(mention,d) 1/softplus(w), w = Zm - zm ----
            w_sb = pp.tile([128, 128], F32, tag="w")
            nc.vector.tensor_tensor(w_sb[:], zmc_sb[:], nzmc_sb[:], OP.add)
            ew = pp.tile([128, 128], F32, tag="ew")
            nc.scalar.activation(ew[:], w_sb[:], AF.Exp)
            spw = pp.tile([128, 128], F32, tag="spw")
            nc.scalar.activation(spw[:], ew[:], AF.Ln, bias=1.0)
            rspw = pp.tile([128, 128], F32, tag="rspw")
            nc.vector.reciprocal(rspw[:], spw[:])

            # ---- sliding ones window for the d-reduction (fp16) ----
            # G[k, 64] = 1 for k < 64 ; G[k, 128] = 1 for k >= 64
            G = pp.tile([128, 192], F16, tag="G")
            nc.vector.memset(G[:], 0.0)
            nc.vector.memset(G[0:64, 64:65], 1.0)
            nc.vector.memset(G[64:128, 128:129], 1.0)

            # ---- main loop ----
            for g in range(2):
                psums = [qp.tile([128, CH], F32, name=f"ps{c}", tag=f"ps{c}") for c in range(NCH)]
                for j in range(64):
                    mp = 64 * g + j
                    b = wp.tile([128, NS], F32, tag="b")
                    u = wp.tile([128, NS], F32, tag="u")
                    nc.vector.tensor_scalar(
                        b[:], Enze[:], Enzm[:, mp:mp + 1], None, OP.min)
                    nc.vector.scalar_tensor_tensor(
                        u[:], EZe[:], EZm[:, mp:mp + 1], b[:],
                        OP.min, OP.mult)
                    sp = ap_.tile([128, NS], F32, tag="sp")
                    nc.scalar.activation(sp[:], u[:], AF.Ln, bias=1.0)
                    lspq = ap_.tile([128, NS], F16, tag="lspq")
                    nc.scalar.activation(lspq[:], sp[:], AF.Ln,
                                         scale=rspw[:, mp:mp + 1])
                    for c in range(NCH):
                        cs = slice(c * CH, (c + 1) * CH)
                        nc.tensor.matmul(
                            psums[c][:], lhsT=G[:, 64 - j:192 - j],
                            rhs=lspq[:, cs], start=(j == 0), stop=(j == 63))
                for c in range(NCH):
                    cs = slice(c * CH, (c + 1) * CH)
                    osb = wp.tile([128, CH], F32, tag="osb")
                    nc.scalar.activation(osb[:], psums[c][:], AF.Exp)
                    nc.sync.dma_start(out=out[g * 128:(g + 1) * 128, cs],
                                      in_=osb[:])
    nc.compile()
    return nc


def _row_perm():
    # psum partition p in group g holds mention 128g+2p (p<64) or
    # 128g+2(p-64)+1 (p>=64); build index: kernel-out row r -> mention index
    perm = np.zeros(M, dtype=np.int64)
    for g in range(2):
        for p in range(128):
            men = 128 * g + (2 * p if p < 64 else 2 * (p - 64) + 1)
            perm[g * 128 + p] = men
    return perm


def _prep_inputs(men_embeds, all_en_embeds):
    men = np.ascontiguousarray(np.asarray(men_embeds, dtype=np.float32))
    en = np.ascontiguousarray(np.asarray(all_en_embeds, dtype=np.float32))
    zm, Zm = men[:, :D], men[:, D:]
    zmc = np.concatenate([Zm[0::2].T, Zm[1::2].T], axis=0)
    nzmc = np.concatenate([-zm[0::2].T, -zm[1::2].T], axis=0)
    zmc = np.ascontiguousarray(zmc, dtype=np.float32)
    nzmc = np.ascontiguousarray(nzmc, dtype=np.float32)
    in_maps = []
    for s in range(NCORES):
        ens = en[s * NS:(s + 1) * NS]
        ze, Ze = ens[:, :D], ens[:, D:]
        zedup = np.ascontiguousarray(
            np.concatenate([Ze.T, Ze.T], axis=0), dtype=np.float32)
        nzedup = np.ascontiguousarray(
            np.concatenate([-ze.T, -ze.T], axis=0), dtype=np.float32)
        in_maps.append({"zedup": zedup, "nzedup": nzedup,
                        "zmc": zmc, "nzmc": nzmc})
    return in_maps


def _run(men_embeds, all_en_embeds, trace=False):
    from concourse.bass_utils import run_bass_kernel_spmd
    if "nc" not in _cache:
        _cache["nc"] = _build()
        _cache["perm"] = _row_perm()
    nc = _cache["nc"]
    in_maps = _prep_inputs(men_embeds, all_en_embeds)
    res = run_bass_kernel_spmd(nc, in_maps, list(range(NCORES)), trace=trace)
    perm = _cache["perm"]
    out = np.empty((M, N), dtype=np.float32)
    for s in range(NCORES):
        block = np.asarray(res.results[s]["out"])
        out[perm, s * NS:(s + 1) * NS] = block
    return out, res


def kernel(men_embeds, all_en_embeds):
    out, _ = _run(men_embeds, all_en_embeds, trace=False)
    return out


def kernel_timed(men_embeds, all_en_embeds):
    out, res = _run(men_embeds, all_en_embeds, trace=True)
    return out, res

